# revision 20
# baseline (speedup 1.0000x reference)
"""Trainium2 Bass kernel for nn_NodeRNN (masked single-step LSTM over N nodes).

Strategy: pure data parallel over the node dim N across 8 cores. All per-node
tensors are staged FEATURE-MAJOR (transposed on host) and converted to BF16
(rel-err budget is 2e-2; bf16 end-to-end lands ~6e-3) so HBM traffic is half
of f32: 2054 B/node vs 4108. Every DMA is contiguous 2KB runs and every
matmul gets its contraction dim on partitions with no on-device transposes.

Per 2048-node tile, split in two 1024-col matmul halves (PSUM bank pair per
matmul, bf16 moving-operand max is 1024):
  x.T   = [W_pos @ [mk;xv].T ; W_hid @ X.T]   PSUM f32          (PE)
  x_sb  = max(x.T + bias_x, 0) -> bf16        relu on GPSIMD
  gates = W_ih @ x.T + W_hh @ hv.T            PSUM f32          (PE)
  i,f,o = sigmoid(g+b), g = tanh(g+b)         -> bf16           (ACT)
  c_new = f*cv + i*g ; h_new = o*tanh(c_new)                    (DVE, bf16 2x)
  inactive rows get old hv/cv copied back     (DVE CP + GPSIMD mask bcast)

The PE stream interleaves gate matmuls of tile t-1 with x matmuls of tile t
so the ACT->PSUM-free dependency never stalls the PE (keeps the HAM clock
warm). Stores trail three iterations so the DMA ring never head-of-line
blocks on compute.
"""
import sys

sys.path.insert(0, "/opt/trn_rl_repo")

import ml_dtypes
import numpy as np

import concourse.bacc as bacc
import concourse.tile as tile
from concourse import mybir
from concourse.bass_utils import run_bass_kernel_spmd

f32 = mybir.dt.float32
bf16 = mybir.dt.bfloat16
i16 = mybir.dt.int16
AF = mybir.ActivationFunctionType
ALU = mybir.AluOpType
npbf16 = ml_dtypes.bfloat16

N = 262144
NCORES = 8
NS = N // NCORES          # 32768 nodes per core
T = 2048                  # nodes per tile (DMA + elementwise granularity)
TH = 1024                 # matmul half-tile (PSUM bank pair, bf16 max cols)
NT = NS // T              # 16 tiles per core
EMBED = 64
EDGE_H = 256
NODE_H = 128
XF = 2 * EDGE_H           # 512 concat(hvv, Hv) features

# xt block: [512, NS] bf16 = [hvv.T; Hv.T] viewed as [128, 4, NS]
# hc block: [256, NS] bf16 = [hv.T; cv.T] viewed as [128, 2, NS]
# aux block: [3, NS] bf16 = [inactive-mask; xv.T]

# bf16 weight block layout: [128, CW], free-dim offsets
CO_WHID = 0               # 4 chunks x 128 cols; cols 64:128 of chunk c
CO_WIH = 512              # W_ih.T [128, 512]
CO_WHH = 1024             # W_hh.T [128, 512]
CO_WP = 1536              # rows 1:3 = W_pos.T [2, 64]; row 0 (mask) zero
CW = 1600
# f32 bias block: [128, 5]: col 0 = concat(b_pos, b_hid); cols 1:5 = gate biases
CB = 5

GATE_FUNCS = [AF.Sigmoid, AF.Sigmoid, AF.Tanh, AF.Sigmoid]  # i, f, g, o

_cached = {}


def build_nc():
    nc = bacc.Bacc(target_bir_lowering=False)
    xt_d = nc.dram_tensor("xt", [XF, NS], bf16, kind="ExternalInput")
    hc_d = nc.dram_tensor("hc", [2 * NODE_H, NS], bf16, kind="ExternalInput")
    aux_d = nc.dram_tensor("aux", [3, NS], bf16, kind="ExternalInput")
    cstw_d = nc.dram_tensor("cstw", [128, CW], bf16, kind="ExternalInput")
    cstb_d = nc.dram_tensor("cstb", [128, CB], f32, kind="ExternalInput")
    out_d = nc.dram_tensor("hc_out", [2 * NODE_H, NS], bf16, kind="ExternalOutput")

    xt_v = xt_d[:].rearrange("(c p) n -> p c n", p=128)    # [128, 4, NS]
    hc_v = hc_d[:].rearrange("(c p) n -> p c n", p=128)    # [128, 2, NS]
    out_v = out_d[:].rearrange("(c p) n -> p c n", p=128)  # [128, 2, NS]

    with tile.TileContext(nc) as tc:
        with (
            tc.tile_pool(name="const", bufs=1) as cpool,
            tc.tile_pool(name="xt", bufs=3) as xtp,
            tc.tile_pool(name="hc", bufs=5) as hcp,
            tc.tile_pool(name="aux", bufs=2) as auxp,
            tc.tile_pool(name="xsb", bufs=2) as xsbp,
            tc.tile_pool(name="msk", bufs=4) as mskp,
            tc.tile_pool(name="gifg", bufs=3) as gifgp,
            tc.tile_pool(name="gao", bufs=3) as gaop,
            tc.tile_pool(name="tmp", bufs=1) as tmpp,
            tc.tile_pool(name="hcn", bufs=3) as hcnp,
            tc.tile_pool(name="ps_x", bufs=2, space="PSUM") as psx,
            tc.tile_pool(name="ps_g", bufs=2, space="PSUM") as psg,
        ):
            cstw = cpool.tile([128, CW], bf16)
            nc.sync.dma_start(cstw[:], cstw_d[:])
            cstb = cpool.tile([128, CB], f32)
            nc.sync.dma_start(cstb[:], cstb_d[:])

            # warmup matmul absorbs the cst DMA wait on the PE
            warm = psx.tile([64, 256], f32, tag="x")
            nc.tensor.matmul(warm[:], cstw[0:3, CO_WP:CO_WP + 64],
                             cstw[0:3, 0:256], start=True, stop=True)

            ld = {}      # t -> (xt_t, hc_t, at, m_sb, x_sb, nsl); dies at S3
            gact_t = {}  # t -> [ga_i, ga_f, ga_g, ga_o]
            x_ps = {}    # t -> [psum half A, psum half B]
            t1_t = {}
            th_t = {}
            hcn_t = {}
            outq = {}

            def emit_loads(t):
                nsl = slice(t * T, (t + 1) * T)
                xt_t = xtp.tile([128, 4, T], bf16, tag="xt")
                nc.sync.dma_start(xt_t[:], xt_v[:, :, nsl])
                hc_t = hcp.tile([128, 2, T], bf16, tag="hc")
                nc.sync.dma_start(hc_t[:], hc_v[:, :, nsl])
                at = auxp.tile([3, T], bf16, tag="aux")
                nc.sync.dma_start(at[:], aux_d[:, nsl])
                m_sb = mskp.tile([128, T], bf16, tag="m")
                nc.gpsimd.partition_broadcast(m_sb[:], at[0:1, :])
                x_sb = xsbp.tile([128, T], bf16, tag="x_sb")
                x_ps[t] = [psx.tile([128, TH], f32, tag="x", name="x_ps_a"),
                           psx.tile([128, TH], f32, tag="x", name="x_ps_b")]
                ld[t] = (xt_t, hc_t, at, m_sb, x_sb, nsl)

            def x_mm(t, h, c):
                """x-matmul for half h, chunk c (c==4 -> aux). Two 512-col
                matmuls (PSUM bank limit) sharing one LDWEIGHTS."""
                xt_t, hc_t, at, m_sb, x_sb, nsl = ld[t]
                xp = x_ps[t][h]
                for s in range(2):
                    ssl = slice(s * 512, (s + 1) * 512)
                    hssl = slice(h * TH + s * 512, h * TH + (s + 1) * 512)
                    if c < 4:
                        nc.tensor.matmul(
                            xp[:, ssl],
                            cstw[:, CO_WHID + 128 * c:CO_WHID + 128 * (c + 1)],
                            xt_t[:, c, hssl], start=(c == 0), stop=False,
                            skip_group_check=True)
                    else:
                        nc.tensor.matmul(xp[0:64, ssl],
                                         cstw[0:3, CO_WP:CO_WP + 64],
                                         at[0:3, hssl], start=False, stop=True,
                                         skip_group_check=True)

            def gate_mm(t, h, j):
                """gate-j matmuls + activation for half h of tile t."""
                xt_t, hc_t, at, m_sb, x_sb, nsl = ld[t]
                hsl = slice(h * TH, (h + 1) * TH)
                ga = gact_t[t][j]
                gp = psg.tile([128, TH], f32, tag="g")
                for s in range(2):
                    ssl = slice(s * 512, (s + 1) * 512)
                    hssl = slice(h * TH + s * 512, h * TH + (s + 1) * 512)
                    nc.tensor.matmul(
                        gp[:, ssl], cstw[:, CO_WHH + 128 * j:CO_WHH + 128 * (j + 1)],
                        hc_t[:, 0, hssl], start=True, stop=False,
                        skip_group_check=True)
                for s in range(2):
                    ssl = slice(s * 512, (s + 1) * 512)
                    hssl = slice(h * TH + s * 512, h * TH + (s + 1) * 512)
                    nc.tensor.matmul(
                        gp[:, ssl], cstw[:, CO_WIH + 128 * j:CO_WIH + 128 * (j + 1)],
                        x_sb[:, hssl], start=False, stop=True,
                        skip_group_check=True)
                nc.scalar.activation(ga[:, hsl], gp[:], GATE_FUNCS[j],
                                     bias=cstb[:, 1 + j:2 + j])

            def emit_relu_act(t):
                """half B relu on ACT (reads PSUM, writes bf16 SBUF)."""
                xt_t, hc_t, at, m_sb, x_sb, nsl = ld[t]
                nc.scalar.activation(x_sb[:, TH:T], x_ps[t][1][:], AF.Relu,
                                     bias=cstb[:, 0:1])

            def emit_relu_dve(t):
                """half A relu on DVE."""
                xt_t, hc_t, at, m_sb, x_sb, nsl = ld[t]
                nc.vector.tensor_scalar(x_sb[:, 0:TH], x_ps[t][0][:],
                                        cstb[:, 0:1], 0.0, ALU.add, ALU.max)

            def emit_t1(t):
                """t1 = f*cv; also allocates hcn(t)."""
                xt_t, hc_t, at, m_sb, x_sb, nsl = ld[t]
                f_s = gact_t[t][1]
                t1 = tmpp.tile([128, T], bf16, tag="t1")
                nc.vector.tensor_tensor(t1[:], f_s[:], hc_t[:, 1, :], ALU.mult)
                t1_t[t] = t1
                hcn_t[t] = hcnp.tile([128, 2, T], bf16, tag="hcn", name="hcn")

            def emit_t2_cadd(t):
                """t2 = i*g, c_new = t1 + t2."""
                i_s, f_s, g_t, o_s = gact_t[t]
                hcn = hcn_t[t]
                t1 = t1_t.pop(t)
                t2 = tmpp.tile([128, T], bf16, tag="t2")
                nc.vector.tensor_tensor(t2[:], i_s[:], g_t[:], ALU.mult)
                nc.vector.tensor_tensor(hcn[:, 1, :], t1[:], t2[:], ALU.add)

            def emit_cpc(t):
                """Last user of ld[t]/gact_t[t]: pops them."""
                xt_t, hc_t, at, m_sb, x_sb, nsl = ld.pop(t)
                gact_t.pop(t)
                hcn = hcn_t.pop(t)
                nc.vector.copy_predicated(hcn[:, 1, :], m_sb[:].bitcast(i16),
                                          hc_t[:, 1, :])

            def emit_tanh(t):
                hcn = hcn_t[t]
                th = tmpp.tile([128, T], bf16, tag="th")
                nc.scalar.activation(th[:], hcn[:, 1, :], AF.Tanh)
                th_t[t] = th

            def emit_hmul_cph(t):
                xt_t, hc_t, at, m_sb, x_sb, nsl = ld[t]
                o_s = gact_t[t][3]
                hcn = hcn_t[t]
                th = th_t.pop(t)
                nc.vector.tensor_tensor(hcn[:, 0, :], o_s[:], th[:], ALU.mult)
                nc.vector.copy_predicated(hcn[:, 0, :], m_sb[:].bitcast(i16),
                                          hc_t[:, 0, :])
                outq[t] = (hcn, ld_nsl[t])

            ld_nsl = {}

            GORD = [1, 0, 2, 3]  # f, i, g, o: f first unblocks t1 earliest
            ND = NT + 4
            for it in range(ND):
                tl = it       # S0 load tile
                tg = it - 1   # S1 gate tile
                tc_ = it - 2  # S2 c_new tile
                tf = it - 3   # S3 finalize tile
                to = it - 4   # S4 store tile
                if 0 <= to < NT:
                    hcn, nsl = outq.pop(to)
                    nc.sync.dma_start(out_v[:, :, nsl], hcn[:])
                if tl < NT:
                    emit_loads(tl)
                    ld_nsl[tl] = slice(tl * T, (tl + 1) * T)
                if 0 <= tg < NT:
                    gact_t[tg] = [
                        gifgp.tile([128, T], bf16, tag="gi", name="ga0"),
                        gifgp.tile([128, T], bf16, tag="gf", name="ga1"),
                        gifgp.tile([128, T], bf16, tag="gg", name="ga2"),
                        gaop.tile([128, T], bf16, tag="go", name="ga3"),
                    ]
                if 0 <= tf < NT:
                    emit_tanh(tf)         # ACT 1: reads pre-mask c' (inactive
                                          # lanes are fixed by cp_h/cp_c later)
                if 0 <= tc_ < NT:
                    emit_t1(tc_)          # DVE 1-3: all inputs finished in
                    emit_t2_cadd(tc_)     # the previous iteration
                # interleaved PE stream: gate mms (tg) | x mms (tl)
                for h in range(2):
                    for jj, j in enumerate(GORD):
                        if 0 <= tg < NT:
                            gate_mm(tg, h, j)
                        if tl < NT:
                            x_mm(tl, h, jj)
                    if tl < NT:
                        x_mm(tl, h, 4)
                    if h == 0:
                        if 0 <= tf < NT:
                            emit_hmul_cph(tf)   # DVE 4,5
                            emit_cpc(tf)        # DVE 6: pure output fixup
                if tl < NT:
                    emit_relu_act(tl)           # ACT last op: half B relu
                    emit_relu_dve(tl)           # DVE last op: half A relu

    nc.finalize()
    return nc


def _stage_inputs(Hv_t, hvv_t, xv_t, hv_tm1, cv_tm1, ts_mask,
                  W_pos, b_pos, W_hid, b_hid, W_ih, b_ih, W_hh, b_hh):
    cstw = np.zeros((128, CW), dtype=npbf16)
    whid_t = np.ascontiguousarray(W_hid.T)          # [512, 64]
    for c in range(4):
        cstw[:, CO_WHID + 128 * c + 64:CO_WHID + 128 * (c + 1)] = \
            whid_t[128 * c:128 * (c + 1)].astype(npbf16)
    cstw[:, CO_WIH:CO_WIH + 512] = W_ih.T.astype(npbf16)
    cstw[:, CO_WHH:CO_WHH + 512] = W_hh.T.astype(npbf16)
    cstw[1:3, CO_WP:CO_WP + 64] = W_pos.T.astype(npbf16)

    cstb = np.zeros((128, CB), dtype=np.float32)
    cstb[:, 0] = np.concatenate([b_pos, b_hid])
    bg = b_ih + b_hh
    cstb[:, 1:5] = bg.reshape(4, 128).T

    # inverted mask: 1.0 where the node is INACTIVE (keeps old state)
    maskf = (ts_mask[:, 0] != 1).astype(npbf16)

    hvv_b = hvv_t.astype(npbf16)
    Hv_b = Hv_t.astype(npbf16)
    hv_b = hv_tm1.astype(npbf16)
    cv_b = cv_tm1.astype(npbf16)
    xv_b = xv_t.astype(npbf16)

    in_maps = []
    for s in range(NCORES):
        sl = slice(s * NS, (s + 1) * NS)
        xt = np.empty((XF, NS), dtype=npbf16)
        xt[0:EDGE_H] = hvv_b[sl].T
        xt[EDGE_H:] = Hv_b[sl].T
        hc = np.empty((2 * NODE_H, NS), dtype=npbf16)
        hc[0:NODE_H] = hv_b[sl].T
        hc[NODE_H:] = cv_b[sl].T
        aux = np.empty((3, NS), dtype=npbf16)
        aux[0] = maskf[sl]
        aux[1:3] = xv_b[sl].T
        in_maps.append(dict(xt=xt, hc=hc, aux=aux, cstw=cstw, cstb=cstb))
    return in_maps


def run(inputs, trace=False):
    """Stage, run on 8 cores, unstage. Returns ((hv_t, cv_t), BassKernelResults)."""
    inputs = {k: np.asarray(v) for k, v in inputs.items()}
    in_maps = _stage_inputs(**inputs)
    if "nc" not in _cached:
        _cached["nc"] = build_nc()
    res = run_bass_kernel_spmd(_cached["nc"], in_maps, core_ids=list(range(NCORES)),
                               trace=trace)
    hv_out = np.empty((N, NODE_H), dtype=np.float32)
    cv_out = np.empty((N, NODE_H), dtype=np.float32)
    for s in range(NCORES):
        sl = slice(s * NS, (s + 1) * NS)
        o = res.results[s]["hc_out"]
        hv_out[sl] = o[0:NODE_H].T.astype(np.float32)
        cv_out[sl] = o[NODE_H:].T.astype(np.float32)
    return (hv_out, cv_out), res


def kernel(**inputs):
    out, _ = run(inputs, trace=False)
    return out


# revision 22
# speedup vs baseline: 1.0565x; 1.0565x over previous
"""Trainium2 Bass kernel for nn_NodeRNN (masked single-step LSTM over N nodes).

Strategy: pure data parallel over the node dim N across 8 cores. All per-node
tensors are staged FEATURE-MAJOR (transposed on host) and converted to BF16
(rel-err budget is 2e-2; bf16 end-to-end lands ~6e-3) so HBM traffic is half
of f32: 2054 B/node vs 4108. Every DMA is contiguous 2KB runs and every
matmul gets its contraction dim on partitions with no on-device transposes.

Per 2048-node tile, split in two 1024-col matmul halves (PSUM bank pair per
matmul, bf16 moving-operand max is 1024):
  x.T   = [W_pos @ [mk;xv].T ; W_hid @ X.T]   PSUM f32          (PE)
  x_sb  = max(x.T + bias_x, 0) -> bf16        relu on GPSIMD
  gates = W_ih @ x.T + W_hh @ hv.T            PSUM f32          (PE)
  i,f,o = sigmoid(g+b), g = tanh(g+b)         -> bf16           (ACT)
  c_new = f*cv + i*g ; h_new = o*tanh(c_new)                    (DVE, bf16 2x)
  inactive rows get old hv/cv copied back     (DVE CP + GPSIMD mask bcast)

The PE stream interleaves gate matmuls of tile t-1 with x matmuls of tile t
so the ACT->PSUM-free dependency never stalls the PE (keeps the HAM clock
warm). Stores trail three iterations so the DMA ring never head-of-line
blocks on compute.
"""
import sys

sys.path.insert(0, "/opt/trn_rl_repo")

import ml_dtypes
import numpy as np

import concourse.bacc as bacc
import concourse.tile as tile
from concourse import mybir
from concourse.bass_utils import run_bass_kernel_spmd

f32 = mybir.dt.float32
bf16 = mybir.dt.bfloat16
i16 = mybir.dt.int16
AF = mybir.ActivationFunctionType
ALU = mybir.AluOpType
npbf16 = ml_dtypes.bfloat16

N = 262144
NCORES = 8
NS = N // NCORES          # 32768 nodes per core
T = 2048                  # nodes per tile (DMA + elementwise granularity)
TH = 1024                 # matmul half-tile (PSUM bank pair, bf16 max cols)
NT = NS // T              # 16 tiles per core
EMBED = 64
EDGE_H = 256
NODE_H = 128
XF = 2 * EDGE_H           # 512 concat(hvv, Hv) features

# xt block: [512, NS] bf16 = [hvv.T; Hv.T] viewed as [128, 4, NS]
# hc block: [256, NS] bf16 = [hv.T; cv.T] viewed as [128, 2, NS]
# aux block: [3, NS] bf16 = [inactive-mask; xv.T]

# bf16 weight block layout: [128, CW], free-dim offsets
CO_WHID = 0               # 4 chunks x 128 cols; cols 64:128 of chunk c
CO_WIH = 512              # W_ih.T [128, 512]
CO_WHH = 1024             # W_hh.T [128, 512]
CO_WP = 1536              # rows 1:3 = W_pos.T [2, 64]; row 0 (mask) zero
CW = 1600
# f32 bias block: [128, 5]: col 0 = concat(b_pos, b_hid); cols 1:5 = gate biases
CB = 5

GATE_FUNCS = [AF.Sigmoid, AF.Sigmoid, AF.Tanh, AF.Sigmoid]  # i, f, g, o

_cached = {}


def build_nc():
    nc = bacc.Bacc(target_bir_lowering=False)
    xt_d = nc.dram_tensor("xt", [XF, NS], bf16, kind="ExternalInput")
    hc_d = nc.dram_tensor("hc", [2 * NODE_H, NS], bf16, kind="ExternalInput")
    aux_d = nc.dram_tensor("aux", [3, NS], bf16, kind="ExternalInput")
    cstw_d = nc.dram_tensor("cstw", [128, CW], bf16, kind="ExternalInput")
    cstb_d = nc.dram_tensor("cstb", [128, CB], f32, kind="ExternalInput")
    out_d = nc.dram_tensor("hc_out", [2 * NODE_H, NS], bf16, kind="ExternalOutput")

    xt_v = xt_d[:].rearrange("(c p) n -> p c n", p=128)    # [128, 4, NS]
    hc_v = hc_d[:].rearrange("(c p) n -> p c n", p=128)    # [128, 2, NS]
    out_v = out_d[:].rearrange("(c p) n -> p c n", p=128)  # [128, 2, NS]

    with tile.TileContext(nc) as tc:
        with (
            tc.tile_pool(name="const", bufs=1) as cpool,
            tc.tile_pool(name="xt", bufs=3) as xtp,
            tc.tile_pool(name="hc", bufs=4) as hcp,
            tc.tile_pool(name="aux", bufs=2) as auxp,
            tc.tile_pool(name="xsb", bufs=2) as xsbp,
            tc.tile_pool(name="msk", bufs=3) as mskp,
            tc.tile_pool(name="gifg", bufs=3) as gifgp,
            tc.tile_pool(name="gao", bufs=3) as gaop,
            tc.tile_pool(name="tmp", bufs=2) as tmpp,
            tc.tile_pool(name="hcn", bufs=3) as hcnp,
            tc.tile_pool(name="ps_x", bufs=2, space="PSUM") as psx,
            tc.tile_pool(name="ps_g", bufs=2, space="PSUM") as psg,
        ):
            cstw = cpool.tile([128, CW], bf16)
            nc.sync.dma_start(cstw[:], cstw_d[:])
            cstb = cpool.tile([128, CB], f32)
            nc.sync.dma_start(cstb[:], cstb_d[:])

            # warmup matmul absorbs the cst DMA wait on the PE
            warm = psx.tile([64, 256], f32, tag="x")
            nc.tensor.matmul(warm[:], cstw[0:3, CO_WP:CO_WP + 64],
                             cstw[0:3, 0:256], start=True, stop=True)

            ld = {}      # t -> (xt_t, hc_t, at, m_sb, x_sb, nsl); dies at S3
            gact_t = {}  # t -> [ga_i, ga_f, ga_g, ga_o]
            x_ps = {}    # t -> [psum half A, psum half B]
            t1_t = {}
            th_t = {}
            hcn_t = {}
            outq = {}

            def emit_loads(t):
                nsl = slice(t * T, (t + 1) * T)
                xt_t = xtp.tile([128, 4, T], bf16, tag="xt")
                nc.sync.dma_start(xt_t[:], xt_v[:, :, nsl])
                hc_t = hcp.tile([128, 2, T], bf16, tag="hc")
                nc.sync.dma_start(hc_t[:], hc_v[:, :, nsl])
                at = auxp.tile([3, T], bf16, tag="aux")
                nc.sync.dma_start(at[:], aux_d[:, nsl])
                m_sb = mskp.tile([128, T], bf16, tag="m")
                nc.gpsimd.partition_broadcast(m_sb[:], at[0:1, :])
                x_sb = xsbp.tile([128, T], bf16, tag="x_sb")
                x_ps[t] = [psx.tile([128, TH], f32, tag="x", name="x_ps_a"),
                           psx.tile([128, TH], f32, tag="x", name="x_ps_b")]
                ld[t] = (xt_t, hc_t, at, m_sb, x_sb, nsl)

            def x_mm(t, h, c):
                """x-matmul for half h, chunk c (c==4 -> aux). Two 512-col
                matmuls (PSUM bank limit) sharing one LDWEIGHTS."""
                xt_t, hc_t, at, m_sb, x_sb, nsl = ld[t]
                xp = x_ps[t][h]
                for s in range(2):
                    ssl = slice(s * 512, (s + 1) * 512)
                    hssl = slice(h * TH + s * 512, h * TH + (s + 1) * 512)
                    if c < 4:
                        nc.tensor.matmul(
                            xp[:, ssl],
                            cstw[:, CO_WHID + 128 * c:CO_WHID + 128 * (c + 1)],
                            xt_t[:, c, hssl], start=(c == 0), stop=False,
                            skip_group_check=True)
                    else:
                        nc.tensor.matmul(xp[0:64, ssl],
                                         cstw[0:3, CO_WP:CO_WP + 64],
                                         at[0:3, hssl], start=False, stop=True,
                                         skip_group_check=True)

            def gate_mm(t, h, j):
                """gate-j matmuls + activation for half h of tile t."""
                xt_t, hc_t, at, m_sb, x_sb, nsl = ld[t]
                hsl = slice(h * TH, (h + 1) * TH)
                ga = gact_t[t][j]
                gp = psg.tile([128, TH], f32, tag="g")
                for s in range(2):
                    ssl = slice(s * 512, (s + 1) * 512)
                    hssl = slice(h * TH + s * 512, h * TH + (s + 1) * 512)
                    nc.tensor.matmul(
                        gp[:, ssl], cstw[:, CO_WHH + 128 * j:CO_WHH + 128 * (j + 1)],
                        hc_t[:, 0, hssl], start=True, stop=False,
                        skip_group_check=True)
                for s in range(2):
                    ssl = slice(s * 512, (s + 1) * 512)
                    hssl = slice(h * TH + s * 512, h * TH + (s + 1) * 512)
                    nc.tensor.matmul(
                        gp[:, ssl], cstw[:, CO_WIH + 128 * j:CO_WIH + 128 * (j + 1)],
                        x_sb[:, hssl], start=False, stop=True,
                        skip_group_check=True)
                nc.scalar.activation(ga[:, hsl], gp[:], GATE_FUNCS[j],
                                     bias=cstb[:, 1 + j:2 + j])

            def emit_relu_act(t):
                """half B relu on ACT (reads PSUM, writes bf16 SBUF)."""
                xt_t, hc_t, at, m_sb, x_sb, nsl = ld[t]
                nc.scalar.activation(x_sb[:, TH:T], x_ps[t][1][:], AF.Relu,
                                     bias=cstb[:, 0:1])

            def emit_relu_dve(t):
                """half A relu on DVE."""
                xt_t, hc_t, at, m_sb, x_sb, nsl = ld[t]
                nc.vector.tensor_scalar(x_sb[:, 0:TH], x_ps[t][0][:],
                                        cstb[:, 0:1], 0.0, ALU.add, ALU.max)

            def emit_t1(t):
                """t1 = f*cv; also allocates hcn(t)."""
                xt_t, hc_t, at, m_sb, x_sb, nsl = ld[t]
                f_s = gact_t[t][1]
                t1 = tmpp.tile([128, T], bf16, tag="t1")
                nc.vector.tensor_tensor(t1[:], f_s[:], hc_t[:, 1, :], ALU.mult)
                t1_t[t] = t1
                hcn_t[t] = hcnp.tile([128, 2, T], bf16, tag="hcn", name="hcn")

            def emit_t2_cadd(t):
                """t2 = i*g, c_new = t1 + t2."""
                i_s, f_s, g_t, o_s = gact_t[t]
                hcn = hcn_t[t]
                t1 = t1_t.pop(t)
                t2 = tmpp.tile([128, T], bf16, tag="t2")
                nc.vector.tensor_tensor(t2[:], i_s[:], g_t[:], ALU.mult)
                nc.vector.tensor_tensor(hcn[:, 1, :], t1[:], t2[:], ALU.add)

            def emit_cpc(t):
                """Last user of ld[t]/gact_t[t]: pops them."""
                xt_t, hc_t, at, m_sb, x_sb, nsl = ld.pop(t)
                gact_t.pop(t)
                hcn = hcn_t.pop(t)
                nc.vector.copy_predicated(hcn[:, 1, :], m_sb[:].bitcast(i16),
                                          hc_t[:, 1, :])

            def emit_tanh(t):
                hcn = hcn_t[t]
                th = tmpp.tile([128, T], bf16, tag="th")
                nc.scalar.activation(th[:], hcn[:, 1, :], AF.Tanh)
                th_t[t] = th

            def emit_hmul_cph(t):
                xt_t, hc_t, at, m_sb, x_sb, nsl = ld[t]
                o_s = gact_t[t][3]
                hcn = hcn_t[t]
                th = th_t.pop(t)
                nc.vector.tensor_tensor(hcn[:, 0, :], o_s[:], th[:], ALU.mult)
                nc.vector.copy_predicated(hcn[:, 0, :], m_sb[:].bitcast(i16),
                                          hc_t[:, 0, :])
                outq[t] = (hcn, ld_nsl[t])

            ld_nsl = {}

            GORD = [1, 0, 2, 3]  # f, i, g, o: f first unblocks t1 earliest
            ND = NT + 4
            for it in range(ND):
                tl = it       # S0 load tile
                tg = it - 1   # S1 gate tile
                tc_ = it - 2  # S2 c_new tile
                tf = it - 3   # S3 finalize tile
                to = it - 4   # S4 store tile
                if 0 <= to < NT:
                    hcn, nsl = outq.pop(to)
                    nc.sync.dma_start(out_v[:, :, nsl], hcn[:])
                if tl < NT:
                    emit_loads(tl)
                    ld_nsl[tl] = slice(tl * T, (tl + 1) * T)
                if 0 <= tg < NT:
                    gact_t[tg] = [
                        gifgp.tile([128, T], bf16, tag="gi", name="ga0"),
                        gifgp.tile([128, T], bf16, tag="gf", name="ga1"),
                        gifgp.tile([128, T], bf16, tag="gg", name="ga2"),
                        gaop.tile([128, T], bf16, tag="go", name="ga3"),
                    ]
                if 0 <= tf < NT:
                    emit_tanh(tf)         # ACT 1: reads pre-mask c' (inactive
                                          # lanes are fixed by cp_h/cp_c later)
                if 0 <= tc_ < NT:
                    emit_t1(tc_)          # DVE 1-3: all inputs finished in
                    emit_t2_cadd(tc_)     # the previous iteration
                # interleaved PE stream: gate mms (tg) | x mms (tl)
                for h in range(2):
                    for jj, j in enumerate(GORD):
                        if 0 <= tg < NT:
                            gate_mm(tg, h, j)
                        if tl < NT:
                            x_mm(tl, h, jj)
                    if tl < NT:
                        x_mm(tl, h, 4)
                    if h == 0:
                        if 0 <= tf < NT:
                            emit_hmul_cph(tf)   # DVE 4,5
                            emit_cpc(tf)        # DVE 6: pure output fixup
                if tl < NT:
                    emit_relu_act(tl)           # ACT last op: half B relu
                    emit_relu_dve(tl)           # DVE last op: half A relu

    nc.finalize()
    return nc


def _stage_inputs(Hv_t, hvv_t, xv_t, hv_tm1, cv_tm1, ts_mask,
                  W_pos, b_pos, W_hid, b_hid, W_ih, b_ih, W_hh, b_hh):
    cstw = np.zeros((128, CW), dtype=npbf16)
    whid_t = np.ascontiguousarray(W_hid.T)          # [512, 64]
    for c in range(4):
        cstw[:, CO_WHID + 128 * c + 64:CO_WHID + 128 * (c + 1)] = \
            whid_t[128 * c:128 * (c + 1)].astype(npbf16)
    cstw[:, CO_WIH:CO_WIH + 512] = W_ih.T.astype(npbf16)
    cstw[:, CO_WHH:CO_WHH + 512] = W_hh.T.astype(npbf16)
    cstw[1:3, CO_WP:CO_WP + 64] = W_pos.T.astype(npbf16)

    cstb = np.zeros((128, CB), dtype=np.float32)
    cstb[:, 0] = np.concatenate([b_pos, b_hid])
    bg = b_ih + b_hh
    cstb[:, 1:5] = bg.reshape(4, 128).T

    # inverted mask: 1.0 where the node is INACTIVE (keeps old state)
    maskf = (ts_mask[:, 0] != 1).astype(npbf16)

    hvv_b = hvv_t.astype(npbf16)
    Hv_b = Hv_t.astype(npbf16)
    hv_b = hv_tm1.astype(npbf16)
    cv_b = cv_tm1.astype(npbf16)
    xv_b = xv_t.astype(npbf16)

    in_maps = []
    for s in range(NCORES):
        sl = slice(s * NS, (s + 1) * NS)
        xt = np.empty((XF, NS), dtype=npbf16)
        xt[0:EDGE_H] = hvv_b[sl].T
        xt[EDGE_H:] = Hv_b[sl].T
        hc = np.empty((2 * NODE_H, NS), dtype=npbf16)
        hc[0:NODE_H] = hv_b[sl].T
        hc[NODE_H:] = cv_b[sl].T
        aux = np.empty((3, NS), dtype=npbf16)
        aux[0] = maskf[sl]
        aux[1:3] = xv_b[sl].T
        in_maps.append(dict(xt=xt, hc=hc, aux=aux, cstw=cstw, cstb=cstb))
    return in_maps


def run(inputs, trace=False):
    """Stage, run on 8 cores, unstage. Returns ((hv_t, cv_t), BassKernelResults)."""
    inputs = {k: np.asarray(v) for k, v in inputs.items()}
    in_maps = _stage_inputs(**inputs)
    if "nc" not in _cached:
        _cached["nc"] = build_nc()
    res = run_bass_kernel_spmd(_cached["nc"], in_maps, core_ids=list(range(NCORES)),
                               trace=trace)
    hv_out = np.empty((N, NODE_H), dtype=np.float32)
    cv_out = np.empty((N, NODE_H), dtype=np.float32)
    for s in range(NCORES):
        sl = slice(s * NS, (s + 1) * NS)
        o = res.results[s]["hc_out"]
        hv_out[sl] = o[0:NODE_H].T.astype(np.float32)
        cv_out[sl] = o[NODE_H:].T.astype(np.float32)
    return (hv_out, cv_out), res


def kernel(**inputs):
    out, _ = run(inputs, trace=False)
    return out
